# revision 1
# baseline (speedup 1.0000x reference)
"""ChainKinematics Trainium2 kernel (8-core data-parallel).

Math per batch element b:
  T_curr_i = offsets[i] @ Rz(theta[b, i])
  abs_i = abs_{i-1} @ T_curr_i           (abs_{-1} = I)
  rel_i = reset_i ? T_curr_i : rel_{i-1} @ T_curr_i

Device mapping (per core, 8192 batch elements):
  State S holds A (4x4 per batch elem) as S[k*32+g, r*256+bw] = A[g*256+bw, r, k]
  (column k on partition blocks of 32, row r in free dim).
  Step: U = A @ O_i on TensorE via block-diag lhsT emitting m-blocks
  [u0, u1, u1, u0] (dup) + [u2, u3]; then the Rz mix on DVE as two
  full products PC = [c*u0 | c*u1], QS = [s*u1 | -s*u0] (the trig tile
  has partition blocks [c, c, s, -s]); GPSIMD adds PC+QS -> new cols 0,1;
  ScalarE copies u2,u3 -> new cols 2,3.  cos/sin computed on device via
  magic-number range reduction + ACT Sin LUT.
"""

import sys

sys.path.insert(0, "/opt/trn_rl_repo")

import numpy as np

N_BODIES = 32
BATCH = 65536
N_CORES = 8
BC = BATCH // N_CORES  # 8192 per core
G = 32  # batch groups (partition blocks)
BW = BC // G  # 256 batch per group
FH = 4 * BW  # 1024: free size of one chain-slot (r, bw)
MAGIC = float(1.5 * 2**23)
TWO_PI = float(2 * np.pi)
INV2PI = float(1.0 / TWO_PI)

_cache = {}


def _build_program(resets):
    """Build the Bass program. resets: sorted tuple of rel-restart bodies (>0)."""
    from concourse import bass, mybir, tile, bacc

    f32 = mybir.dt.float32
    f32r = mybir.dt.float32r

    split = resets[0] if resets else N_BODIES  # first dual body

    nc = bacc.Bacc(None, target_bir_lowering=False, debug=False)
    threp_d = nc.dram_tensor("threp", [128, BC], f32, kind="ExternalInput")
    wall_d = nc.dram_tensor("wall", [128, N_BODIES * 192], f32r, kind="ExternalInput")
    wsum_d = nc.dram_tensor("wsum", [128, 64], f32r, kind="ExternalInput")
    oabs_d = nc.dram_tensor("oabs", [N_BODIES, 128, FH], f32r, kind="ExternalOutput")
    orel_d = nc.dram_tensor(
        "orel", [N_BODIES - split, 128, FH], f32r, kind="ExternalOutput"
    )

    with tile.TileContext(nc) as tc:
        with (
            tc.tile_pool(name="wpool", bufs=1) as wpool,
            tc.tile_pool(name="trigpool", bufs=1) as trigpool,
            tc.tile_pool(name="cpool", bufs=1) as cpool,
        ):
            w_tile = wpool.tile([128, N_BODIES * 192], f32r)
            nc.sync.dma_start(w_tile[:], wall_d[:])
            wsum = wpool.tile([128, 64], f32r)
            nc.sync.dma_start(wsum[:], wsum_d[:])
            trig = trigpool.tile([128, BC], f32)

            # per-partition constants: blocks [c, c, s, -s]
            m_b = cpool.tile([128, 1], f32)
            scl = cpool.tile([128, 1], f32)
            bias = cpool.tile([128, 1], f32)
            nc.vector.memset(m_b[0:64, :], 0.25)
            nc.vector.memset(m_b[64:128, :], 0.0)
            nc.vector.memset(scl[0:96, :], 1.0)
            nc.vector.memset(scl[96:128, :], -1.0)
            nc.vector.memset(bias[0:64, :], float(np.pi / 2))
            nc.vector.memset(bias[64:128, :], 0.0)

            # ---- trig phase (scratch freed afterwards) ----
            # body-major free layout: f = i*BW + bw. Computed in chunks so the
            # chain scan can start as soon as the first bodies' trig is ready.
            with tc.tile_pool(name="scratch", bufs=2) as sp:
                threp = trigpool.tile([128, BC], f32, tag="threp")
                nc.sync.dma_start(threp[:], threp_d[:])
                bounds = [0, 2 * BW, 8 * BW, BC]
                for lo, hi in zip(bounds[:-1], bounds[1:]):
                    sl = slice(lo, hi)
                    n = hi - lo
                    y1 = sp.tile([128, n], f32, tag="y")
                    nc.vector.tensor_scalar(
                        y1[:], threp[:, sl], INV2PI, m_b[:, 0:1],
                        mybir.AluOpType.mult, mybir.AluOpType.add,
                    )
                    y2 = sp.tile([128, n], f32, tag="y")
                    nc.vector.tensor_scalar(
                        y2[:], y1[:], MAGIC, None, mybir.AluOpType.add
                    )
                    y3 = sp.tile([128, n], f32, tag="y")
                    nc.vector.tensor_scalar(
                        y3[:], y2[:], MAGIC, None, mybir.AluOpType.subtract
                    )
                    y4 = sp.tile([128, n], f32, tag="y")
                    nc.vector.scalar_tensor_tensor(
                        y4[:], y3[:], -TWO_PI, threp[:, sl],
                        mybir.AluOpType.mult, mybir.AluOpType.add,
                    )
                    nc.scalar.activation(
                        trig[:, sl], y4[:], mybir.ActivationFunctionType.Sin,
                        bias=bias[:, 0:1], scale=scl[:, 0:1],
                    )

            # ---- state phase ----
            with (
                tc.tile_pool(name="spool", bufs=6) as spool,
                tc.tile_pool(name="idpool", bufs=1) as idpool,
                tc.tile_pool(name="mixpool", bufs=10) as mixpool,
                tc.tile_pool(name="u2pool", bufs=3, space=bass.MemorySpace.PSUM) as u2pool,
                tc.tile_pool(name="u23pool", bufs=2, space=bass.MemorySpace.PSUM) as u23pool,
                tc.tile_pool(name="sumpool", bufs=2, space=bass.MemorySpace.PSUM) as sumpool,
            ):
                sid_f = idpool.tile([128, FH], f32)
                nc.vector.memset(sid_f[:], 0.0)
                for k in range(4):
                    nc.vector.memset(
                        sid_f[k * 32 : (k + 1) * 32, k * BW : (k + 1) * BW], 1.0
                    )
                sid = idpool.tile([128, FH], f32r)
                nc.vector.tensor_copy(sid[:], sid_f[:])

                s_prev = None
                nsub = [0]
                for i in range(N_BODIES):
                    dual = i >= split
                    s_next = spool.tile([128, 2 * FH], f32r, tag="state")
                    slots = [0, 1] if dual else [0]
                    for slot in slots:
                        if i == 0 or (slot == 1 and i in resets):
                            rhs = sid[:]
                        elif slot == 1 and i == split:
                            # first dual body: rel restarts at split, so this
                            # branch is covered by the reset case above
                            rhs = sid[:]
                        else:
                            # rel before split equals abs (slot 0 of s_prev)
                            off = FH if (slot == 1 and i > split) else 0
                            rhs = s_prev[:, off : off + FH]
                        fo = slot * FH  # free offset in s_next
                        wd = w_tile[:, i * 192 : i * 192 + 128]
                        w2 = w_tile[:, i * 192 + 128 : i * 192 + 192]
                        # split single-chain bodies into two independent free
                        # sub-halves (r in {0,1} and r in {2,3}) to deepen
                        # the PE->DVE->POOL/ACT pipeline; dual bodies already
                        # have 2-way chain parallelism so keep ops full-width
                        SUB = 512
                        for sub in range(0, FH, SUB):
                            nr = SUB // BW  # r-values in this sub-slot
                            u2 = u2pool.tile([128, SUB], mybir.dt.float32, tag="u2")
                            u23 = u23pool.tile([64, SUB], mybir.dt.float32, tag="u23")
                            csz = min(512, SUB)
                            for ch in range(0, SUB, csz):
                                ms = slice(sub + ch, sub + ch + csz)
                                us = slice(ch, ch + csz)
                                nc.tensor.matmul(
                                    u2[:, us], wd, rhs[:, ms], start=True, stop=True
                                )
                                nc.tensor.matmul(
                                    u23[:, us], w2, rhs[:, ms], start=True, stop=True
                                )
                            tsl = slice(i * BW, (i + 1) * BW)
                            tb = (
                                trig[:, tsl]
                                .unsqueeze(1)
                                .broadcast_to([128, nr, BW])
                            )
                            pq = mixpool.tile([128, SUB], f32r, tag="pq")
                            nc.vector.tensor_mul(
                                pq[:].rearrange("p (r b) -> p r b", b=BW),
                                u2[:].rearrange("p (r b) -> p r b", b=BW),
                                tb,
                            )
                            c01 = sumpool.tile([64, SUB], mybir.dt.float32, tag="c01")
                            nc.tensor.matmul(
                                c01[:], wsum[:], pq[:], start=True, stop=True
                            )
                            nsub[0] += 1
                            if nsub[0] % 2 == 0:
                                nc.vector.tensor_copy(
                                    s_next[0:64, fo + sub : fo + sub + SUB], c01[:]
                                )
                            else:
                                nc.scalar.copy(
                                    s_next[0:64, fo + sub : fo + sub + SUB], c01[:]
                                )
                            nc.scalar.copy(
                                s_next[64:128, fo + sub : fo + sub + SUB], u23[:]
                            )
                        if slot == 0:
                            nc.sync.dma_start(oabs_d[i, :, :], s_next[:, 0:FH])
                        else:
                            nc.sync.dma_start(
                                orel_d[i - split, :, :], s_next[:, FH : 2 * FH]
                            )
                    s_prev = s_next

    nc.compile()
    return nc, split


def kernel(theta, offsets, reset_mask):
    theta = np.asarray(theta, dtype=np.float32)
    offsets = np.asarray(offsets, dtype=np.float32)
    reset_mask = np.asarray(reset_mask)
    assert theta.shape == (BATCH, N_BODIES)
    assert bool(reset_mask[0]), "chain must reset at body 0"
    resets = tuple(int(i) for i in np.flatnonzero(reset_mask) if i > 0)

    from concourse.bass_utils import run_bass_kernel_spmd
    import os

    key = resets
    if key not in _cache:
        _cache[key] = _build_program(resets)
    nc, split = _cache[key]

    # block-sum lhsT: col0 = PQ0 + PQ2, col1 = PQ1 + PQ3
    W_sum = np.zeros((128, 64), np.float32)
    for q, j in [(0, 0), (2, 0), (1, 1), (3, 1)]:
        W_sum[q * G + np.arange(G), j * G + np.arange(G)] = 1.0
    # host-prepared weights: per body, lhsT blocks for [u0,u1,u1,u0] and [u2,u3]
    W_all = np.zeros((128, N_BODIES * 192), np.float32)
    gidx = np.arange(G)
    for i in range(N_BODIES):
        O = offsets[i]
        for k in range(4):
            for mb, j in enumerate([0, 1, 1, 0]):
                W_all[k * G + gidx, i * 192 + mb * G + gidx] = O[k, j]
            for mb, j in enumerate([2, 3]):
                W_all[k * G + gidx, i * 192 + 128 + mb * G + gidx] = O[k, j]

    # host-prepared theta: [128, BC] with partition blocks [c,c,s,-s] all equal
    # to theta in layout [g, (bw, i)]; value th[g*BW+bw, i] at (q*32+g, bw*32+i)
    in_maps = []
    for c in range(N_CORES):
        thc = theta[c * BC : (c + 1) * BC]  # [8192, 32]
        th_g = np.ascontiguousarray(
            thc.reshape(G, BW, N_BODIES).transpose(0, 2, 1).reshape(G, BW * N_BODIES)
        )  # [32, 8192]
        threp = np.tile(th_g, (4, 1))  # [128, 8192]
        in_maps.append({"threp": threp, "wall": W_all, "wsum": W_sum})

    out = run_bass_kernel_spmd(nc, in_maps, core_ids=list(range(N_CORES)))
    kernel.last_exec_ns = out.exec_time_ns
    kernel.last_results = out

    def decode(arr):
        # [nb, 128, FH] -> [nb, BC, 4, 4]: p=(k,g), f=(r,bw)
        nb = arr.shape[0]
        a = arr.reshape(nb, 4, G, 4, BW)  # i, k, g, r, bw
        return np.ascontiguousarray(
            a.transpose(0, 2, 4, 3, 1).reshape(nb, BC, 4, 4)
        )

    abs_full = np.empty((N_BODIES, BATCH, 4, 4), np.float32)
    rel_full = np.empty((N_BODIES, BATCH, 4, 4), np.float32)
    for c in range(N_CORES):
        res = out.results[c]
        bsl = slice(c * BC, (c + 1) * BC)
        abs_full[:, bsl] = decode(res["oabs"])
        rel_full[split:, bsl] = decode(res["orel"])
    rel_full[:split] = abs_full[:split]
    return abs_full, rel_full


kernel.last_exec_ns = None
kernel.last_results = None



# revision 2
# speedup vs baseline: 1.1644x; 1.1644x over previous
"""ChainKinematics Trainium2 kernel (8-core data-parallel), v2.

Math per batch element b:
  T_curr_i = offsets[i] @ Rz(theta[b, i])
  abs_i = abs_{i-1} @ T_curr_i           (abs_{-1} = I)
  rel_i = reset_i ? T_curr_i : rel_{i-1} @ T_curr_i

Layout (per core, 8192 batch elements, fp16 state):
  State S[k*32+g, r*256+bw] = A[g*256+bw, r, k]  (col k on partition
  blocks of 32, row r in free dim).  Step, per chain slot:
    U  = wd_i^T @ S            (PE; m-blocks [u0,u1,u1,u0], PSUM f32)
    X[64:128] = w2_i^T @ S     (PE; m-blocks [u2,u3] at base 64)
    pq = U * trig_i            (DVE; trig q-blocks [c,c,s,-s]; fp16 SBUF)
    X[0:64] = wsum^T @ pq      (PE; block sum -> [col0, col1])
    s_next = copy(X)           (ACT; one f32 PSUM -> fp16 SBUF copy)
  Trig via magic-number range reduction (y1-y3 on GPSIMD, y4 on DVE)
  + ACT Sin with per-partition scale/bias, computed in the replicated
  [q,g] x (i,bw) layout directly from a replicated theta input.
"""

import sys

sys.path.insert(0, "/opt/trn_rl_repo")

import numpy as np

N_BODIES = 32
BATCH = 65536
N_CORES = 8
BC = BATCH // N_CORES  # 8192 per core
G = 32  # batch groups (partition blocks)
BW = BC // G  # 256 batch per group
FH = 4 * BW  # 1024: free size of one chain-slot (r, bw)
SUB = FH // 2  # 512: r-split sub-slot for single-chain bodies
MAGIC = float(1.5 * 2**23)
PI = float(np.pi)
TWO_PI = float(2 * PI)
INV2PI = float(1.0 / TWO_PI)

# trig chunk bounds over the [i, bw] free dim (i-major, 256 per body)
TRIG_BOUNDS = [0, 256, 512, 1024, 2048, 3072, 4096, 6144, 8192]
# issue chunk k's ops before this body index
TRIG_ISSUE_BODY = [0, 0, 0, 2, 4, 6, 8, 12]

_cache = {}


def _build_program(resets):
    """Build the Bass program. resets: sorted tuple of rel-restart bodies (>0)."""
    from concourse import bass, mybir, tile, bacc

    f32 = mybir.dt.float32
    f16 = mybir.dt.float16

    split = resets[0] if resets else N_BODIES  # first dual body

    nc = bacc.Bacc(None, target_bir_lowering=False, debug=False)
    threp_d = nc.dram_tensor("threp", [128, BC], f32, kind="ExternalInput")
    wall_d = nc.dram_tensor("wall", [128, N_BODIES * 192], f16, kind="ExternalInput")
    wsum_d = nc.dram_tensor("wsum", [128, 64], f16, kind="ExternalInput")
    oabs_d = nc.dram_tensor("oabs", [N_BODIES, 128, FH], f16, kind="ExternalOutput")
    orel_d = nc.dram_tensor(
        "orel", [N_BODIES - split, 128, FH], f16, kind="ExternalOutput"
    )

    with tile.TileContext(nc) as tc:
        with (
            tc.tile_pool(name="wpool", bufs=1) as wpool,
            tc.tile_pool(name="trigpool", bufs=1) as trigpool,
            tc.tile_pool(name="cpool", bufs=1) as cpool,
            tc.tile_pool(name="thpool", bufs=2) as thpool,
            tc.tile_pool(name="ypool", bufs=2) as ypool,
            tc.tile_pool(name="spool", bufs=4) as spool,
            tc.tile_pool(name="idpool", bufs=1) as idpool,
            tc.tile_pool(name="pqpool", bufs=3) as pqpool,
            tc.tile_pool(name="upool", bufs=2, space=bass.MemorySpace.PSUM) as upool,
            tc.tile_pool(name="xpool", bufs=2, space=bass.MemorySpace.PSUM) as xpool,
        ):
            # ---- weights ----
            w_tile = wpool.tile([128, N_BODIES * 192], f16)
            nc.sync.dma_start(w_tile[:, 0 : split * 192], wall_d[:, 0 : split * 192])
            nc.sync.dma_start(
                w_tile[:, split * 192 :], wall_d[:, split * 192 :]
            )
            wsum = wpool.tile([128, 64], f16)
            nc.sync.dma_start(wsum[:], wsum_d[:])

            # ---- per-partition trig constants ----
            m_b = cpool.tile([128, 1], f32)  # cos blocks offset 0.25
            nc.vector.memset(m_b[0:64, :], 0.25)
            nc.vector.memset(m_b[64:128, :], 0.0)
            scl = cpool.tile([128, 1], f32)  # sin scale: [1,1,1,-1]
            nc.vector.memset(scl[0:96, :], 1.0)
            nc.vector.memset(scl[96:128, :], -1.0)
            bias = cpool.tile([128, 1], f32)  # sin bias: [pi/2, pi/2, 0, 0]
            nc.vector.memset(bias[0:64, :], float(PI / 2))
            nc.vector.memset(bias[64:128, :], 0.0)
            one = cpool.tile([128, 1], f32)
            nc.vector.memset(one[:, :], 1.0)
            mag = cpool.tile([128, 1], f32)
            nc.vector.memset(mag[:, :], MAGIC)
            nmag = cpool.tile([128, 1], f32)
            nc.vector.memset(nmag[:, :], -MAGIC)

            trig = trigpool.tile([128, BC], f16)

            def trig_chunk(lo, hi):
                n = hi - lo
                th = thpool.tile([128, n], f32, tag=f"th{n}")
                nc.sync.dma_start(th[:], threp_d[:, lo:hi])
                y1 = ypool.tile([128, n], f32, tag=f"y1_{n}")
                nc.gpsimd.tensor_scalar(
                    y1[:], th[:], INV2PI, m_b[:, 0:1],
                    mybir.AluOpType.mult, mybir.AluOpType.add,
                )
                y2 = ypool.tile([128, n], f32, tag=f"y2_{n}")
                nc.gpsimd.tensor_scalar(
                    y2[:], y1[:], one[:, 0:1], mag[:, 0:1],
                    mybir.AluOpType.mult, mybir.AluOpType.add,
                )
                y3 = ypool.tile([128, n], f32, tag=f"y3_{n}")
                nc.gpsimd.tensor_scalar(
                    y3[:], y2[:], one[:, 0:1], nmag[:, 0:1],
                    mybir.AluOpType.mult, mybir.AluOpType.add,
                )
                y4 = ypool.tile([128, n], f32, tag=f"y4_{n}")
                nc.vector.scalar_tensor_tensor(
                    y4[:], y3[:], -TWO_PI, th[:],
                    mybir.AluOpType.mult, mybir.AluOpType.add,
                )
                nc.scalar.activation(
                    trig[:, lo:hi], y4[:], mybir.ActivationFunctionType.Sin,
                    bias=bias[:, 0:1], scale=scl[:, 0:1],
                )

            n_chunks = len(TRIG_BOUNDS) - 1
            next_chunk = [0]

            def issue_trig(body):
                while (
                    next_chunk[0] < n_chunks
                    and TRIG_ISSUE_BODY[next_chunk[0]] <= body
                ):
                    k = next_chunk[0]
                    trig_chunk(TRIG_BOUNDS[k], TRIG_BOUNDS[k + 1])
                    next_chunk[0] += 1

            # ---- identity initial state (fp16) ----
            sid = idpool.tile([128, FH], f16)
            nc.vector.memset(sid[:], 0.0)
            for k in range(4):
                nc.vector.memset(
                    sid[k * 32 : (k + 1) * 32, k * BW : (k + 1) * BW], 1.0
                )

            issue_trig(0)

            def do_sub(rhs, wd, w2, s_next, fo, tsl, lo, hi):
                """One pipeline unit: state transform for free range [lo:hi)
                of a chain slot. rhs: [128, FH] AP of prev state; writes
                s_next[:, fo+lo : fo+hi]."""
                n = hi - lo
                nr = n // BW
                U = upool.tile([128, FH], mybir.dt.float32, tag="u")
                X = xpool.tile([128, FH], mybir.dt.float32, tag="x")
                for ch in range(lo, hi, SUB):
                    ce = min(ch + SUB, hi)
                    nc.tensor.matmul(
                        U[:, ch:ce], wd, rhs[:, ch:ce], start=True, stop=True
                    )
                    nc.tensor.matmul(
                        X[64:128, ch:ce], w2, rhs[:, ch:ce], start=True, stop=True
                    )
                tb = trig[:, tsl].unsqueeze(1).broadcast_to([128, nr, BW])
                pq = pqpool.tile([128, FH], f16, tag="pq")
                nc.vector.tensor_mul(
                    pq[:, lo:hi].rearrange("p (r b) -> p r b", b=BW),
                    U[:, lo:hi].rearrange("p (r b) -> p r b", b=BW),
                    tb,
                )
                for ch in range(lo, hi, SUB):
                    ce = min(ch + SUB, hi)
                    nc.tensor.matmul(
                        X[0:64, ch:ce], wsum[:], pq[:, ch:ce], start=True, stop=True
                    )
                nc.scalar.copy(s_next[:, fo + lo : fo + hi], X[:, lo:hi])

            s_prev = None
            for i in range(N_BODIES):
                issue_trig(i)
                dual = i >= split
                s_next = spool.tile([128, 2 * FH], f16, tag="state")
                wd = w_tile[:, i * 192 : i * 192 + 128]
                w2 = w_tile[:, i * 192 + 128 : i * 192 + 192]
                tsl = slice(i * BW, (i + 1) * BW)
                for slot in [0, 1] if dual else [0]:
                    if i == 0 or (slot == 1 and i in resets):
                        rhs = sid[:]
                    else:
                        off = FH if (slot == 1 and i > split) else 0
                        rhs = s_prev[:, off : off + FH]
                    fo = slot * FH
                    if dual:
                        do_sub(rhs, wd, w2, s_next, fo, tsl, 0, FH)
                        if slot == 0:
                            nc.sync.dma_start(oabs_d[i, :, :], s_next[:, 0:FH])
                        else:
                            nc.sync.dma_start(
                                orel_d[i - split, :, :], s_next[:, FH : 2 * FH]
                            )
                    else:
                        # r-split: two independent sub-chains for pipelining
                        tsl2 = slice(i * BW, (i + 1) * BW)
                        for lo in range(0, FH, SUB):
                            do_sub(rhs, wd, w2, s_next, fo, tsl2, lo, lo + SUB)
                            nc.sync.dma_start(
                                oabs_d[i, :, lo : lo + SUB],
                                s_next[:, lo : lo + SUB],
                            )
                s_prev = s_next

    nc.compile()
    return nc, split


def kernel(theta, offsets, reset_mask):
    theta = np.asarray(theta, dtype=np.float32)
    offsets = np.asarray(offsets, dtype=np.float32)
    reset_mask = np.asarray(reset_mask)
    assert theta.shape == (BATCH, N_BODIES)
    assert bool(reset_mask[0]), "chain must reset at body 0"
    resets = tuple(int(i) for i in np.flatnonzero(reset_mask) if i > 0)

    from concourse.bass_utils import run_bass_kernel_spmd

    key = resets
    if key not in _cache:
        _cache[key] = _build_program(resets)
    nc, split = _cache[key]

    # block-sum lhsT: col0 = PQ0 + PQ2, col1 = PQ1 + PQ3
    W_sum = np.zeros((128, 64), np.float16)
    gidx = np.arange(G)
    for q, j in [(0, 0), (2, 0), (1, 1), (3, 1)]:
        W_sum[q * G + gidx, j * G + gidx] = 1.0
    # per body: lhsT blocks for [u0,u1,u1,u0] (128 cols) and [u2,u3] (64 cols)
    W_all = np.zeros((128, N_BODIES * 192), np.float16)
    for i in range(N_BODIES):
        O = offsets[i]
        for k in range(4):
            for mb, j in enumerate([0, 1, 1, 0]):
                W_all[k * G + gidx, i * 192 + mb * G + gidx] = O[k, j]
            for mb, j in enumerate([2, 3]):
                W_all[k * G + gidx, i * 192 + 128 + mb * G + gidx] = O[k, j]

    # replicated theta: [128, BC]; value th[g*BW+bw, i] at (q*32+g, i*BW+bw)
    in_maps = []
    for c in range(N_CORES):
        thc = theta[c * BC : (c + 1) * BC]  # [8192, 32]
        th_g = np.ascontiguousarray(
            thc.reshape(G, BW, N_BODIES).transpose(0, 2, 1).reshape(G, N_BODIES * BW)
        )  # [32, 8192] laid out (i, bw)
        threp = np.tile(th_g, (4, 1))  # [128, 8192]
        in_maps.append({"threp": threp, "wall": W_all, "wsum": W_sum})

    out = run_bass_kernel_spmd(nc, in_maps, core_ids=list(range(N_CORES)))
    kernel.last_exec_ns = out.exec_time_ns
    kernel.last_results = out

    def decode(arr):
        # [nb, 128, FH] -> [nb, BC, 4, 4]: p=(k,g), f=(r,bw)
        nb = arr.shape[0]
        a = arr.astype(np.float32).reshape(nb, 4, G, 4, BW)  # i, k, g, r, bw
        return np.ascontiguousarray(
            a.transpose(0, 2, 4, 3, 1).reshape(nb, BC, 4, 4)
        )

    abs_full = np.empty((N_BODIES, BATCH, 4, 4), np.float32)
    rel_full = np.empty((N_BODIES, BATCH, 4, 4), np.float32)
    for c in range(N_CORES):
        res = out.results[c]
        bsl = slice(c * BC, (c + 1) * BC)
        abs_full[:, bsl] = decode(res["oabs"])
        rel_full[split:, bsl] = decode(res["orel"])
    rel_full[:split] = abs_full[:split]
    return abs_full, rel_full


kernel.last_exec_ns = None
kernel.last_results = None


# revision 3
# speedup vs baseline: 1.6001x; 1.3742x over previous
"""ChainKinematics Trainium2 kernel (8-core data-parallel), v3.

Math per batch element b:
  T_curr_i = offsets[i] @ Rz(theta[b, i])
  abs_i = abs_{i-1} @ T_curr_i           (abs_{-1} = I)
  rel_i = reset_i ? T_curr_i : rel_{i-1} @ T_curr_i

Layout (per core, 8192 batch elements, fp16 state):
  State S[k*32+g, r*256+bw] = A[g*256+bw, r, k].  Every chain slot is
  r-split into two independent 512-wide sub-chains (r in {0,1} / {2,3}).
  Per sub:
    U  = wd_i^T @ S            (PE; m-blocks [u0,u1,u1,u0], PSUM f32)
    X[64:128] = w2_i^T @ S     (PE; m-blocks [u2,u3] at partition base 64)
    pq = U * trig_i            (DVE; trig q-blocks [c,c,s,-s]; fp16 SBUF)
    X[0:64] = wsum^T @ pq      (PE; block sum -> [col0, col1])
    s_next = copy(X)           (ACT; one f32 PSUM -> fp16 SBUF copy)
  Trig via magic-number range reduction (y1-y3 on GPSIMD, y4 on DVE)
  + ACT Sin with per-partition scale/bias, in the replicated
  [q,g] x (i,bw) layout.  All input DMAs are issued upfront so the SP
  DMA queue never head-of-line blocks chain-dependent input loads.
"""

import sys

sys.path.insert(0, "/opt/trn_rl_repo")

import numpy as np

N_BODIES = 32
BATCH = 65536
N_CORES = 8
BC = BATCH // N_CORES  # 8192 per core
G = 32  # batch groups (partition blocks)
BW = BC // G  # 256 batch per group
FH = 4 * BW  # 1024: free size of one chain-slot (r, bw)
SUB = FH // 2  # 512: r-split sub-slot
MAGIC = float(1.5 * 2**23)
PI = float(np.pi)
TWO_PI = float(2 * PI)
INV2PI = float(1.0 / TWO_PI)

# trig compute chunks over the (i, bw) free dim and the body index
# before which each chunk's compute is issued
TRIG_BOUNDS = [0, 256, 512, 1024, 2048, 3072, 4096, 6144, 8192]
TRIG_ISSUE_BODY = [0, 0, 0, 2, 6, 9, 12, 18]
# upfront input DMA chunks for threp
TH_DMA_BOUNDS = [0, 256, 2048, 8192]

_cache = {}


def _build_program(resets):
    from concourse import bass, mybir, tile, bacc

    f32 = mybir.dt.float32
    f16 = mybir.dt.float16

    split = resets[0] if resets else N_BODIES  # first dual body

    nc = bacc.Bacc(None, target_bir_lowering=False, debug=False)
    threp_d = nc.dram_tensor("threp", [128, BC], f32, kind="ExternalInput")
    wall_d = nc.dram_tensor("wall", [128, N_BODIES * 192], f16, kind="ExternalInput")
    wsum_d = nc.dram_tensor("wsum", [128, 64], f16, kind="ExternalInput")
    oabs_d = nc.dram_tensor("oabs", [N_BODIES, 128, FH], f16, kind="ExternalOutput")
    orel_d = nc.dram_tensor(
        "orel", [N_BODIES - split, 128, FH], f16, kind="ExternalOutput"
    )

    with tile.TileContext(nc) as tc:
        with (
            tc.tile_pool(name="wpool", bufs=1) as wpool,
            tc.tile_pool(name="trigpool", bufs=1) as trigpool,
            tc.tile_pool(name="cpool", bufs=1) as cpool,
            tc.tile_pool(name="ypool", bufs=2) as ypool,
            tc.tile_pool(name="spool", bufs=4) as spool,
            tc.tile_pool(name="idpool", bufs=1) as idpool,
            tc.tile_pool(name="pqpool", bufs=6) as pqpool,
            tc.tile_pool(name="upool", bufs=4, space=bass.MemorySpace.PSUM) as upool,
            tc.tile_pool(name="xpool", bufs=4, space=bass.MemorySpace.PSUM) as xpool,
        ):
            # ---- upfront input DMAs (no waits; SP queue stays unblocked) ----
            w_tile = wpool.tile([128, N_BODIES * 192], f16)
            nc.sync.dma_start(w_tile[:, 0 : split * 192], wall_d[:, 0 : split * 192])
            nc.sync.dma_start(w_tile[:, split * 192 :], wall_d[:, split * 192 :])
            wsum = wpool.tile([128, 64], f16)
            nc.sync.dma_start(wsum[:], wsum_d[:])
            threp = trigpool.tile([128, BC], f32)
            for lo, hi in zip(TH_DMA_BOUNDS[:-1], TH_DMA_BOUNDS[1:]):
                nc.sync.dma_start(threp[:, lo:hi], threp_d[:, lo:hi])

            # ---- per-partition trig constants ----
            m_b = cpool.tile([128, 1], f32)  # cos blocks offset 0.25
            nc.vector.memset(m_b[0:64, :], 0.25)
            nc.vector.memset(m_b[64:128, :], 0.0)
            scl = cpool.tile([128, 1], f32)  # sin scale: [1,1,1,-1]
            nc.vector.memset(scl[0:96, :], 1.0)
            nc.vector.memset(scl[96:128, :], -1.0)
            bias = cpool.tile([128, 1], f32)  # sin bias: [pi/2, pi/2, 0, 0]
            nc.vector.memset(bias[0:64, :], float(PI / 2))
            nc.vector.memset(bias[64:128, :], 0.0)
            one = cpool.tile([128, 1], f32)
            nc.vector.memset(one[:, :], 1.0)
            mag = cpool.tile([128, 1], f32)
            nc.vector.memset(mag[:, :], MAGIC)
            nmag = cpool.tile([128, 1], f32)
            nc.vector.memset(nmag[:, :], -MAGIC)

            trig = trigpool.tile([128, BC], f16)

            def trig_chunk(lo, hi):
                n = hi - lo
                y1 = ypool.tile([128, n], f32, tag=f"y1_{n}")
                nc.gpsimd.tensor_scalar(
                    y1[:], threp[:, lo:hi], INV2PI, m_b[:, 0:1],
                    mybir.AluOpType.mult, mybir.AluOpType.add,
                )
                y2 = ypool.tile([128, n], f32, tag=f"y2_{n}")
                nc.gpsimd.tensor_scalar(
                    y2[:], y1[:], one[:, 0:1], mag[:, 0:1],
                    mybir.AluOpType.mult, mybir.AluOpType.add,
                )
                y3 = ypool.tile([128, n], f32, tag=f"y3_{n}")
                nc.gpsimd.tensor_scalar(
                    y3[:], y2[:], one[:, 0:1], nmag[:, 0:1],
                    mybir.AluOpType.mult, mybir.AluOpType.add,
                )
                y4 = ypool.tile([128, n], f32, tag=f"y4_{n}")
                nc.vector.scalar_tensor_tensor(
                    y4[:], y3[:], -TWO_PI, threp[:, lo:hi],
                    mybir.AluOpType.mult, mybir.AluOpType.add,
                )
                nc.scalar.activation(
                    trig[:, lo:hi], y4[:], mybir.ActivationFunctionType.Sin,
                    bias=bias[:, 0:1], scale=scl[:, 0:1],
                )

            n_chunks = len(TRIG_BOUNDS) - 1
            next_chunk = [0]

            def issue_trig(body):
                while (
                    next_chunk[0] < n_chunks
                    and TRIG_ISSUE_BODY[next_chunk[0]] <= body
                ):
                    k = next_chunk[0]
                    trig_chunk(TRIG_BOUNDS[k], TRIG_BOUNDS[k + 1])
                    next_chunk[0] += 1

            # ---- identity initial state (fp16) ----
            sid = idpool.tile([128, FH], f16)
            nc.vector.memset(sid[:], 0.0)
            for k in range(4):
                nc.vector.memset(
                    sid[k * 32 : (k + 1) * 32, k * BW : (k + 1) * BW], 1.0
                )

            s_prev = None
            for i in range(N_BODIES):
                issue_trig(i)
                dual = i >= split
                s_next = spool.tile([128, 2 * FH], f16, tag="state")
                wd = w_tile[:, i * 192 : i * 192 + 128]
                w2 = w_tile[:, i * 192 + 128 : i * 192 + 192]

                # sub-units: (slot, lo) for each r-split half of each slot
                units = []
                for slot in [0, 1] if dual else [0]:
                    if i == 0 or (slot == 1 and i in resets):
                        rhs = sid[:]
                    else:
                        off = FH if (slot == 1 and i > split) else 0
                        rhs = s_prev[:, off : off + FH]
                    for lo in (0, SUB):
                        units.append((slot, rhs, lo))

                # phase A: state-transform matmuls (independent per sub)
                us, xs = [], []
                for slot, rhs, lo in units:
                    U = upool.tile([128, SUB], mybir.dt.float32, tag="u")
                    X = xpool.tile([128, SUB], mybir.dt.float32, tag="x")
                    sl = slice(lo, lo + SUB)
                    nc.tensor.matmul(U[:], wd, rhs[:, sl], start=True, stop=True)
                    nc.tensor.matmul(
                        X[64:128, :], w2, rhs[:, sl], start=True, stop=True
                    )
                    us.append(U)
                    xs.append(X)

                # phase B: trig mixes (DVE)
                tb = (
                    trig[:, i * BW : (i + 1) * BW]
                    .unsqueeze(1)
                    .broadcast_to([128, 2, BW])
                )
                pqs = []
                for (slot, rhs, lo), U in zip(units, us):
                    pq = pqpool.tile([128, SUB], f16, tag="pq")
                    nc.vector.tensor_mul(
                        pq[:].rearrange("p (r b) -> p r b", b=BW),
                        U[:].rearrange("p (r b) -> p r b", b=BW),
                        tb,
                    )
                    pqs.append(pq)

                # phase C: block-sum matmuls
                for (slot, rhs, lo), X, pq in zip(units, xs, pqs):
                    nc.tensor.matmul(
                        X[0:64, :], wsum[:], pq[:], start=True, stop=True
                    )

                # phase D: PSUM -> fp16 state copies (ACT)
                for (slot, rhs, lo), X in zip(units, xs):
                    nc.scalar.copy(
                        s_next[:, slot * FH + lo : slot * FH + lo + SUB], X[:]
                    )

                # outputs (one DMA per slot)
                nc.sync.dma_start(oabs_d[i, :, :], s_next[:, 0:FH])
                if dual:
                    nc.sync.dma_start(
                        orel_d[i - split, :, :], s_next[:, FH : 2 * FH]
                    )
                s_prev = s_next

    nc.compile()
    return nc, split


def kernel(theta, offsets, reset_mask):
    theta = np.asarray(theta, dtype=np.float32)
    offsets = np.asarray(offsets, dtype=np.float32)
    reset_mask = np.asarray(reset_mask)
    assert theta.shape == (BATCH, N_BODIES)
    assert bool(reset_mask[0]), "chain must reset at body 0"
    resets = tuple(int(i) for i in np.flatnonzero(reset_mask) if i > 0)

    from concourse.bass_utils import run_bass_kernel_spmd

    key = resets
    if key not in _cache:
        _cache[key] = _build_program(resets)
    nc, split = _cache[key]

    # block-sum lhsT: col0 = PQ0 + PQ2, col1 = PQ1 + PQ3
    W_sum = np.zeros((128, 64), np.float16)
    gidx = np.arange(G)
    for q, j in [(0, 0), (2, 0), (1, 1), (3, 1)]:
        W_sum[q * G + gidx, j * G + gidx] = 1.0
    # per body: lhsT blocks for [u0,u1,u1,u0] (128 cols) and [u2,u3] (64 cols)
    W_all = np.zeros((128, N_BODIES * 192), np.float16)
    for i in range(N_BODIES):
        O = offsets[i]
        for k in range(4):
            for mb, j in enumerate([0, 1, 1, 0]):
                W_all[k * G + gidx, i * 192 + mb * G + gidx] = O[k, j]
            for mb, j in enumerate([2, 3]):
                W_all[k * G + gidx, i * 192 + 128 + mb * G + gidx] = O[k, j]

    # replicated theta: [128, BC]; value th[g*BW+bw, i] at (q*32+g, i*BW+bw)
    in_maps = []
    for c in range(N_CORES):
        thc = theta[c * BC : (c + 1) * BC]  # [8192, 32]
        th_g = np.ascontiguousarray(
            thc.reshape(G, BW, N_BODIES).transpose(0, 2, 1).reshape(G, N_BODIES * BW)
        )  # [32, 8192] laid out (i, bw)
        threp = np.tile(th_g, (4, 1))  # [128, 8192]
        in_maps.append({"threp": threp, "wall": W_all, "wsum": W_sum})

    out = run_bass_kernel_spmd(nc, in_maps, core_ids=list(range(N_CORES)))
    kernel.last_exec_ns = out.exec_time_ns
    kernel.last_results = out

    def decode(arr):
        # [nb, 128, FH] -> [nb, BC, 4, 4]: p=(k,g), f=(r,bw)
        nb = arr.shape[0]
        a = arr.astype(np.float32).reshape(nb, 4, G, 4, BW)  # i, k, g, r, bw
        return np.ascontiguousarray(
            a.transpose(0, 2, 4, 3, 1).reshape(nb, BC, 4, 4)
        )

    abs_full = np.empty((N_BODIES, BATCH, 4, 4), np.float32)
    rel_full = np.empty((N_BODIES, BATCH, 4, 4), np.float32)
    for c in range(N_CORES):
        res = out.results[c]
        bsl = slice(c * BC, (c + 1) * BC)
        abs_full[:, bsl] = decode(res["oabs"])
        rel_full[split:, bsl] = decode(res["orel"])
    rel_full[:split] = abs_full[:split]
    return abs_full, rel_full


kernel.last_exec_ns = None
kernel.last_results = None
